# revision 14
# baseline (speedup 1.0000x reference)
"""Gated multi-head self-attention on 8 Trainium2 NeuronCores.

Sharding: batch (B=2) x head-groups (4 groups of 4 heads) -> 8 cores.
Each core computes, for its batch b and its 4 heads:
    partial_out[t, e] = sum_h gate[h] * softmax(Q_h K_h^T / 8) V_h Wo_h
The host sums the 4 head-group partials per batch, adds the constant
term sum_h gate_h*(bo_h + bv_h Wo_h) (bv/bo commute past the softmax
normalization), and stacks the two batches.

v2 design (ACT exp is the critical path: 8 groups x 16 x [128,1024]
exps ~= 141us/core):
  - all matmul inputs bf16 (halves DMA, enables FWL weight loads);
    scores themselves accumulate in fp32 PSUM so softmax is accurate
  - no K=1 bias matmuls: bq/bk added during the DVE eviction of Q/K
    (per-partition scalar add), bv/bo folded into a host-side constant
  - rowsum: DVE adds ex tiles into 4 partial sums, PE accumulates the
    partials via [128,2]-ones stationary matmuls -> [2,512] PSUM row
    per head, reciprocal_approx_fast, one sel2 broadcast matmul
  - attention groups pr-outer; scores/exp stream ahead, PV lags by 2;
    V-proj, remaining Q/K projections and outproj are emitted after the
    group that needs them next, so the Tile scheduler (priority =
    emission order) runs them in PE slack under the ACT-bound groups
"""

import numpy as np
import ml_dtypes
from contextlib import ExitStack

import concourse.bass as bass
import concourse.tile as tile
from concourse import bacc, mybir
from concourse import bass_utils

E, H, D = 1024, 16, 64
B, T = 2, 2048
NCORES = 8
P = 128
TC = 512          # t-chunk (PSUM bank = 512 fp32)
NTC = T // TC     # 4 t-chunks
NST = T // P      # 16 s-tiles
NEC = E // P      # 8 e-chunks

F32 = mybir.dt.float32
F32R = mybir.dt.float32r
BF16 = mybir.dt.bfloat16
ADD = mybir.AluOpType.add
MULT = mybir.AluOpType.mult


def build_kernel():
    nc = bacc.Bacc("TRN2", target_bir_lowering=False, debug=False,
                   num_devices=NCORES)
    hT = nc.dram_tensor("hT", [NEC, P, T], BF16, kind="ExternalInput").ap()
    wq = nc.dram_tensor("wq", [P, 2, NEC, P], BF16, kind="ExternalInput").ap()
    wk = nc.dram_tensor("wk", [P, 2, NEC, P], BF16, kind="ExternalInput").ap()
    wv = nc.dram_tensor("wv", [P, NEC, 256], BF16, kind="ExternalInput").ap()
    wo = nc.dram_tensor("wo", [P, 2, E], BF16, kind="ExternalInput").ap()
    bq = nc.dram_tensor("bq", [P, 2], F32, kind="ExternalInput").ap()
    bk = nc.dram_tensor("bk", [P, 2], F32, kind="ExternalInput").ap()
    on2 = nc.dram_tensor("on2", [P, 4], BF16, kind="ExternalInput").ap()
    sel2 = nc.dram_tensor("sel2", [2, P], BF16, kind="ExternalInput").ap()
    out = nc.dram_tensor("out", [T, E], F32, kind="ExternalOutput").ap()

    with tile.TileContext(nc) as tc:
        with ExitStack() as ctx:
            persist = ctx.enter_context(tc.tile_pool(name="persist", bufs=1))
            expool = ctx.enter_context(tc.tile_pool(name="expool", bufs=20))
            rspool = ctx.enter_context(tc.tile_pool(name="rspool", bufs=2))
            rcpool = ctx.enter_context(tc.tile_pool(name="rcpool", bufs=2))
            work = ctx.enter_context(tc.tile_pool(name="work", bufs=4))
            ps_s = ctx.enter_context(tc.tile_pool(name="ps_s", bufs=2, space="PSUM"))
            ps_ctx = ctx.enter_context(tc.tile_pool(name="ps_ctx", bufs=2, space="PSUM"))
            ps_misc = ctx.enter_context(tc.tile_pool(name="ps_misc", bufs=2, space="PSUM"))

            # ---- persistent SBUF tensors ----
            hT_sb = persist.tile([P, NEC, T], BF16, tag="hT")
            wq_sb = persist.tile([P, 2, NEC, P], BF16, tag="wq")
            wk_sb = persist.tile([P, 2, NEC, P], BF16, tag="wk")
            wv_sb = persist.tile([P, NEC, 256], BF16, tag="wv")
            wo_sb = persist.tile([P, 2, E], BF16, tag="wo")
            bq_sb = persist.tile([P, 2], F32, tag="bq")
            bk_sb = persist.tile([P, 2], F32, tag="bk")
            on2_sb = persist.tile([P, 4], BF16, tag="on2")
            sel2_sb = persist.tile([2, P], BF16, tag="sel2")
            QT_sb = persist.tile([P, 2, T], BF16, tag="QT")
            KT_sb = persist.tile([P, 2, T], BF16, tag="KT")
            V_sb = persist.tile([P, NST, 256], BF16, tag="V")
            ctx_sb = persist.tile([P, 2, T], BF16, tag="ctx")

            with nc.named_scope("load"):
                # PE warm-up: ~24 dummy matmuls on uninitialized SBUF keep the
                # tensor engine busy during the input DMAs so the HAM clock
                # gate reaches 8/8 (2.4 GHz) before the real work starts.
                for w in range(8):
                    psw = ps_misc.tile([P, TC], F32, tag="ps_misc",
                                       name=f"warm{w}")
                    nc.tensor.matmul(psw[:], KT_sb[0:64, 0, 0:P],
                                     QT_sb[0:64, 0, 0:TC],
                                     start=True, stop=True)
                nc.sync.dma_start(wk_sb[:], wk)
                nc.sync.dma_start(wq_sb[:], wq)
                nc.sync.dma_start(bq_sb[:], bq)
                nc.sync.dma_start(bk_sb[:], bk)
                nc.sync.dma_start(on2_sb[:], on2)
                nc.sync.dma_start(sel2_sb[:], sel2)
                for ec in range(NEC):
                    for sp in range(4):
                        nc.sync.dma_start(
                            hT_sb[sp * 32:(sp + 1) * 32, ec, :],
                            hT[ec][sp * 32:(sp + 1) * 32])
                nc.sync.dma_start(wv_sb[:], wv)
                nc.sync.dma_start(wo_sb[:], wo)

            def proj_qk(w_sb, b_sb, dst, pr, tch):
                """One [128, TC] chunk of the Q or K projection (+bias)."""
                ps = ps_misc.tile([P, TC], F32, tag="ps_misc")
                for ec in range(NEC):
                    nc.tensor.matmul(
                        ps[:], w_sb[:, pr, ec, :],
                        hT_sb[:, ec, tch * TC:(tch + 1) * TC],
                        start=(ec == 0), stop=(ec == NEC - 1))
                nc.vector.tensor_scalar(
                    dst[:, pr, tch * TC:(tch + 1) * TC], ps[:],
                    b_sb[:, pr:pr + 1], None, ADD)

            def proj_v(st):
                """V rows for s-tile st: [128 t, 256 d] -> V_sb bf16."""
                ps = ps_misc.tile([P, TC], F32, tag="ps_misc")
                psv = ps[:, :256]
                for ec in range(NEC):
                    nc.tensor.matmul(
                        psv, hT_sb[:, ec, st * P:(st + 1) * P],
                        wv_sb[:, ec, :], start=(ec == 0), stop=(ec == NEC - 1))
                nc.vector.tensor_copy(V_sb[:, st, :], psv)

            def outproj(tt):
                """Output projection for t-tile tt (128 t) -> HBM."""
                for ec2 in range(2):
                    pso = ps_misc.tile([P, TC], F32, tag="ps_misc")
                    for pr in range(2):
                        nc.tensor.matmul(
                            pso[:], ctx_sb[:, pr, tt * P:(tt + 1) * P],
                            wo_sb[:, pr, ec2 * TC:(ec2 + 1) * TC],
                            start=(pr == 0), stop=(pr == 1))
                    o_sb = work.tile([P, TC], F32, tag="o")
                    nc.vector.tensor_copy(o_sb[:], pso[:])
                    nc.sync.dma_start(
                        out[tt * P:(tt + 1) * P, ec2 * TC:(ec2 + 1) * TC],
                        o_sb[:])

            def pv(pctx, ex, st, pr):
                c0 = pr * P
                nc.tensor.matmul(
                    pctx[0:64, :], V_sb[:, st, c0:c0 + 64], ex[:, :TC],
                    start=(st == 0), stop=(st == NST - 1),
                    tile_position=(0, 0), skip_group_check=True)
                nc.tensor.matmul(
                    pctx[64:P, :], V_sb[:, st, c0 + 64:c0 + P], ex[:, TC:],
                    start=(st == 0), stop=(st == NST - 1),
                    tile_position=(0, 64), skip_group_check=True)

            with nc.named_scope("qkv"):
                # K(t0)/Q(t0) interleaved per e-chunk: each pair of matmuls
                # only needs hT chunk ec, so the PE tracks the chunk DMAs
                psk = ps_misc.tile([P, TC], F32, tag="ps_misc", name="psk0")
                psq = ps_misc.tile([P, TC], F32, tag="ps_misc", name="psq0")
                for ec in range(NEC):
                    nc.tensor.matmul(
                        psk[:], wk_sb[:, 0, ec, :], hT_sb[:, ec, 0:TC],
                        start=(ec == 0), stop=(ec == NEC - 1),
                        skip_group_check=True)
                    nc.tensor.matmul(
                        psq[:], wq_sb[:, 0, ec, :], hT_sb[:, ec, 0:TC],
                        start=(ec == 0), stop=(ec == NEC - 1),
                        skip_group_check=True)
                nc.vector.tensor_scalar(
                    KT_sb[:, 0, 0:TC], psk[:], bk_sb[:, 0:1], None, ADD)
                nc.vector.tensor_scalar(
                    QT_sb[:, 0, 0:TC], psq[:], bq_sb[:, 0:1], None, ADD)

            # work quanta interleaved one-per-iteration into the attention
            # groups as (min_iteration, fn); emission order = semantic order
            # AND scheduler priority.  K(t1..3) are just-in-time inside
            # group 0 (scores of s-tile 4*t need K(t), first used at
            # iteration 4*t); V is split across groups 0-1 ahead of the
            # lag-4 PV consumers; outproj(tt of tch) waits for the ctx
            # scale of group 4+tch, spilled to iteration 6 of group 5+tch.
            def QK(wb, bb, dstb, pr, t):
                return lambda: proj_qk(wb, bb, dstb, pr, t)

            extras = {
                0: [(0, QK(wk_sb, bk_sb, KT_sb, 0, 1)),
                    (1, QK(wk_sb, bk_sb, KT_sb, 0, 2)),
                    (2, QK(wk_sb, bk_sb, KT_sb, 0, 3)),
                    (3, QK(wq_sb, bq_sb, QT_sb, 0, 1))]
                   + [(4 + j, (lambda st=st: proj_v(st)))
                      for j, st in enumerate(range(0, 12))],
                1: [(j, (lambda st=st: proj_v(st)))
                    for j, st in enumerate(range(12, NST))]
                   + [(4, QK(wq_sb, bq_sb, QT_sb, 0, 2)),
                      (8, QK(wk_sb, bk_sb, KT_sb, 1, 0))],
                2: [(0, QK(wq_sb, bq_sb, QT_sb, 0, 3)),
                    (4, QK(wk_sb, bk_sb, KT_sb, 1, 1)),
                    (8, QK(wk_sb, bk_sb, KT_sb, 1, 2))],
                3: [(0, QK(wk_sb, bk_sb, KT_sb, 1, 3)),
                    (6, QK(wq_sb, bq_sb, QT_sb, 1, 0))],
                4: [(0, QK(wq_sb, bq_sb, QT_sb, 1, 1))],
                5: [(0, QK(wq_sb, bq_sb, QT_sb, 1, 2))]
                   + [(7 + 2 * j, (lambda tt=tt: outproj(tt)))
                      for j, tt in enumerate(range(0, 4))],
                6: [(0, QK(wq_sb, bq_sb, QT_sb, 1, 3))]
                   + [(7 + 2 * j, (lambda tt=tt: outproj(tt)))
                      for j, tt in enumerate(range(4, 8))],
                7: [(7 + 2 * j, (lambda tt=tt: outproj(tt)))
                    for j, tt in enumerate(range(8, 12))],
            }

            # ---- attention: 8 groups, pr-outer ----
            # Group 0's PV matmuls and tail are deferred into group 1 so
            # V-proj (interleaved through group 0) has a full group of
            # margin; later groups run PV with a 2-iteration lag.
            def tail_pieces(gi, tch, pr, pctx, rs):
                t0 = tch * TC
                state = {}

                def p_rowsum():
                    prs = ps_misc.tile([P, TC], F32, tag="ps_misc",
                                       name=f"prs_{gi}")
                    nc.tensor.matmul(prs[0:2, :], on2_sb[:, 0:2], rs[:, :TC],
                                     start=True, stop=False)
                    nc.tensor.matmul(prs[0:2, :], on2_sb[:, 2:4], rs[:, TC:],
                                     start=False, stop=True)
                    rcp = rcpool.tile([2, TC], F32, tag="rcp")
                    nc.vector.reciprocal_approx_fast(rcp[:], prs[0:2, :])
                    rcp_bf = rcpool.tile([2, TC], BF16, tag="rcpb")
                    nc.vector.tensor_copy(rcp_bf[:], rcp[:])
                    state["rcp_bf"] = rcp_bf

                def p_bcast():
                    pR = ps_misc.tile([P, TC], F32, tag="ps_misc",
                                      name=f"pR_{gi}")
                    nc.tensor.matmul(pR[:], sel2_sb[:], state["rcp_bf"][:],
                                     start=True, stop=True)
                    state["pR"] = pR

                def p_mult():
                    R_sb = work.tile([P, TC], F32, tag="R", name=f"R_{gi}")
                    nc.vector.tensor_copy(R_sb[:], state["pR"][:])
                    nc.vector.tensor_tensor(
                        ctx_sb[:, pr, t0:t0 + TC], pctx[:], R_sb[:], MULT)

                return [p_rowsum, p_bcast, p_mult]

            with nc.named_scope("attn"):
                groups = [(tch, pr) for pr in range(2) for tch in range(NTC)]
                LAG = 4
                spill = []   # prev group leftovers: PV 12..15 + tail pieces
                for gi, (tch, pr) in enumerate(groups):
                    t0 = tch * TC
                    quota = sorted(extras.get(gi, []), key=lambda x: x[0])
                    qi = 0
                    pctx = ps_ctx.tile([P, TC], F32, tag="ps_ctx")
                    rs = rspool.tile([P, 2 * TC], BF16, tag="rs",
                                     name=f"rs_{gi}")
                    exs = [None] * NST
                    for st in range(NST):
                        s0 = st * P
                        pss = ps_s.tile([P, 2 * TC], F32, tag="ps_s")
                        nc.tensor.matmul(
                            pss[:, :TC], KT_sb[0:64, pr, s0:s0 + P],
                            QT_sb[0:64, pr, t0:t0 + TC],
                            start=True, stop=True, tile_position=(0, 0))
                        nc.tensor.matmul(
                            pss[:, TC:], KT_sb[64:P, pr, s0:s0 + P],
                            QT_sb[64:P, pr, t0:t0 + TC],
                            start=True, stop=True, tile_position=(64, 0))
                        ex = expool.tile([P, 2 * TC], BF16, tag="expT")
                        exs[st] = ex
                        nc.scalar.activation(
                            ex[:], pss[:],
                            mybir.ActivationFunctionType.Exp, scale=0.125)
                        if st == 1:
                            nc.vector.tensor_tensor(
                                rs[:], exs[0][:], ex[:], ADD)
                        elif st > 1:
                            nc.vector.tensor_tensor(rs[:], rs[:], ex[:], ADD)
                        # one extra-work quantum per iteration
                        while qi < len(quota) and quota[qi][0] <= st:
                            quota[qi][1]()
                            qi += 1
                            break
                        # one prev-group spill op per iteration
                        if st < len(spill):
                            spill[st]()
                        # this group's PV, lagged so its exp wait and V-proj
                        # are long satisfied when the PE reaches it
                        if st >= LAG:
                            pv(pctx, exs[st - LAG], st - LAG, pr)
                    while qi < len(quota):
                        quota[qi][1]()
                        qi += 1
                    spill = [
                        (lambda s=s, pc=pctx, e=exs[s], p=pr: pv(pc, e, s, p))
                        for s in range(NST - LAG, NST)
                    ] + tail_pieces(gi, tch, pr, pctx, rs)
                # last group's leftovers
                for fn in spill:
                    fn()

            with nc.named_scope("outproj"):
                for tt in range(12, NST):
                    outproj(tt)
    nc.compile()
    return nc


_NC = None


def _get_nc():
    global _NC
    if _NC is None:
        _NC = build_kernel()
    return _NC


def make_in_maps(hidden_states, Wq, bq, Wk, bk, Wv, bv, Wo, bo, gate):
    f = np.float32
    b16 = ml_dtypes.bfloat16
    hidden_states = np.asarray(hidden_states, f)
    Wq, bq = np.asarray(Wq, f), np.asarray(bq, f)
    Wk, bk = np.asarray(Wk, f), np.asarray(bk, f)
    Wv, bv = np.asarray(Wv, f), np.asarray(bv, f)
    Wo, bo = np.asarray(Wo, f), np.asarray(bo, f)
    gate = np.asarray(gate, f)

    hT_b = [np.ascontiguousarray(hidden_states[b].T)
            .reshape(NEC, P, T).astype(b16) for b in range(B)]
    on2_np = np.zeros((P, 4), b16)
    on2_np[:, 0] = 1.0   # head-A rowsum -> psum row 0
    on2_np[:, 3] = 1.0   # head-B rowsum -> psum row 1
    sel2_np = np.zeros((2, P), b16)
    sel2_np[0, 0:64] = 1.0
    sel2_np[1, 64:P] = 1.0

    in_maps = []
    consts = []
    for core in range(NCORES):
        b, hg = divmod(core, 4)
        hs = [4 * hg + i for i in range(4)]

        def pack_qk(W):
            outw = np.empty((P, 2, NEC, P), f)
            for pr in range(2):
                pair = np.concatenate(
                    [W[hs[2 * pr]], W[hs[2 * pr + 1]]], axis=1)  # [E, 128]
                outw[:, pr] = pair.reshape(NEC, P, P).transpose(1, 0, 2)
            return outw.astype(b16)

        wv_np = np.concatenate([Wv[h] for h in hs], axis=1)  # [E, 256]
        wv_np = wv_np.reshape(NEC, P, 256).transpose(1, 0, 2).astype(b16)
        wo_np = np.empty((2, P, E), f)
        bq_np = np.empty((P, 2), f)
        bk_np = np.empty((P, 2), f)
        for pr in range(2):
            h0, h1 = hs[2 * pr], hs[2 * pr + 1]
            wo_np[pr] = np.concatenate(
                [gate[h0] * Wo[h0], gate[h1] * Wo[h1]], axis=0)  # [128, E]
            bq_np[:, pr] = np.concatenate([bq[h0], bq[h1]])
            bk_np[:, pr] = np.concatenate([bk[h0], bk[h1]])
        # constant term: sum_h gate_h * (bo_h + bv_h @ Wo_h)   [E]
        cst = sum(gate[h] * (bo[h] + bv[h] @ Wo[h]) for h in hs)
        consts.append(np.asarray(cst, f))
        in_maps.append(dict(
            hT=np.ascontiguousarray(hT_b[b]),
            wq=np.ascontiguousarray(pack_qk(Wq)),
            wk=np.ascontiguousarray(pack_qk(Wk)),
            wv=np.ascontiguousarray(wv_np),
            wo=np.ascontiguousarray(wo_np.transpose(1, 0, 2).astype(b16)),
            bq=bq_np, bk=bk_np,
            on2=on2_np, sel2=sel2_np,
        ))
    return in_maps, consts


def kernel(hidden_states, Wq, bq, Wk, bk, Wv, bv, Wo, bo, gate, _trace=False,
           **run_kwargs):
    nc = _get_nc()
    in_maps, consts = make_in_maps(
        hidden_states, Wq, bq, Wk, bk, Wv, bv, Wo, bo, gate)
    res = bass_utils.run_bass_kernel_spmd(
        nc, in_maps, core_ids=list(range(NCORES)), trace=_trace, **run_kwargs)
    outs = [r["out"] for r in res.results]
    full = np.stack([
        outs[0] + outs[1] + outs[2] + outs[3]
        + (consts[0] + consts[1] + consts[2] + consts[3])[None, :],
        outs[4] + outs[5] + outs[6] + outs[7]
        + (consts[4] + consts[5] + consts[6] + consts[7])[None, :],
    ]).astype(np.float32)
    kernel.last_result = res
    return full


# revision 16
# speedup vs baseline: 1.0275x; 1.0275x over previous
"""Gated multi-head self-attention on 8 Trainium2 NeuronCores.

Sharding: batch (B=2) x head-groups (4 groups of 4 heads) -> 8 cores.
Each core computes, for its batch b and its 4 heads:
    partial_out[t, e] = sum_h gate[h] * softmax(Q_h K_h^T / 8) V_h Wo_h
The host sums the 4 head-group partials per batch, adds the constant
term sum_h gate_h*(bo_h + bv_h Wo_h) (bv/bo commute past the softmax
normalization), and stacks the two batches.

v2 design (ACT exp is the critical path: 8 groups x 16 x [128,1024]
exps ~= 141us/core):
  - all matmul inputs bf16 (halves DMA, enables FWL weight loads);
    scores themselves accumulate in fp32 PSUM so softmax is accurate
  - no K=1 bias matmuls: bq/bk added during the DVE eviction of Q/K
    (per-partition scalar add), bv/bo folded into a host-side constant
  - rowsum: DVE adds ex tiles into 4 partial sums, PE accumulates the
    partials via [128,2]-ones stationary matmuls -> [2,512] PSUM row
    per head, reciprocal_approx_fast, one sel2 broadcast matmul
  - attention groups pr-outer; scores/exp stream ahead, PV lags by 2;
    V-proj, remaining Q/K projections and outproj are emitted after the
    group that needs them next, so the Tile scheduler (priority =
    emission order) runs them in PE slack under the ACT-bound groups
"""

import numpy as np
import ml_dtypes
from contextlib import ExitStack

import concourse.bass as bass
import concourse.tile as tile
from concourse import bacc, mybir
from concourse import bass_utils

E, H, D = 1024, 16, 64
B, T = 2, 2048
NCORES = 8
P = 128
TC = 512          # t-chunk (PSUM bank = 512 fp32)
NTC = T // TC     # 4 t-chunks
NST = T // P      # 16 s-tiles
NEC = E // P      # 8 e-chunks

F32 = mybir.dt.float32
F32R = mybir.dt.float32r
BF16 = mybir.dt.bfloat16
ADD = mybir.AluOpType.add
MULT = mybir.AluOpType.mult


def build_kernel():
    nc = bacc.Bacc("TRN2", target_bir_lowering=False, debug=False,
                   num_devices=NCORES)
    hT = nc.dram_tensor("hT", [NEC, P, T], BF16, kind="ExternalInput").ap()
    wq = nc.dram_tensor("wq", [P, 2, NEC, P], BF16, kind="ExternalInput").ap()
    wk = nc.dram_tensor("wk", [P, 2, NEC, P], BF16, kind="ExternalInput").ap()
    wv = nc.dram_tensor("wv", [P, NEC, 256], BF16, kind="ExternalInput").ap()
    wo = nc.dram_tensor("wo", [P, 2, E], BF16, kind="ExternalInput").ap()
    bq = nc.dram_tensor("bq", [P, 2], F32, kind="ExternalInput").ap()
    bk = nc.dram_tensor("bk", [P, 2], F32, kind="ExternalInput").ap()
    on2 = nc.dram_tensor("on2", [P, 4], BF16, kind="ExternalInput").ap()
    sel2 = nc.dram_tensor("sel2", [2, P], BF16, kind="ExternalInput").ap()
    out = nc.dram_tensor("out", [T, E], F32, kind="ExternalOutput").ap()

    with tile.TileContext(nc) as tc:
        with ExitStack() as ctx:
            persist = ctx.enter_context(tc.tile_pool(name="persist", bufs=1))
            expool = ctx.enter_context(tc.tile_pool(name="expool", bufs=20))
            rspool = ctx.enter_context(tc.tile_pool(name="rspool", bufs=2))
            rcpool = ctx.enter_context(tc.tile_pool(name="rcpool", bufs=2))
            work = ctx.enter_context(tc.tile_pool(name="work", bufs=4))
            ps_s = ctx.enter_context(tc.tile_pool(name="ps_s", bufs=2, space="PSUM"))
            ps_ctx = ctx.enter_context(tc.tile_pool(name="ps_ctx", bufs=2, space="PSUM"))
            ps_misc = ctx.enter_context(tc.tile_pool(name="ps_misc", bufs=2, space="PSUM"))

            # ---- persistent SBUF tensors ----
            hT_sb = persist.tile([P, NEC, T], BF16, tag="hT")
            wq_sb = persist.tile([P, 2, NEC, P], BF16, tag="wq")
            wk_sb = persist.tile([P, 2, NEC, P], BF16, tag="wk")
            wv_sb = persist.tile([P, NEC, 256], BF16, tag="wv")
            wo_sb = persist.tile([P, 2, E], BF16, tag="wo")
            bq_sb = persist.tile([P, 2], F32, tag="bq")
            bk_sb = persist.tile([P, 2], F32, tag="bk")
            on2_sb = persist.tile([P, 4], BF16, tag="on2")
            sel2_sb = persist.tile([2, P], BF16, tag="sel2")
            QT_sb = persist.tile([P, 2, T], BF16, tag="QT")
            KT_sb = persist.tile([P, 2, T], BF16, tag="KT")
            V_sb = persist.tile([P, NST, 256], BF16, tag="V")
            ctx_sb = persist.tile([P, 2, T], BF16, tag="ctx")

            with nc.named_scope("load"):
                # PE warm-up matmuls on uninitialized SBUF keep the tensor
                # engine busy during the input DMAs so the HAM clock gate
                # ramps toward 8/8 before the real work starts.
                for w in range(8):
                    psw = ps_misc.tile([P, TC], F32, tag="ps_misc",
                                       name=f"warm{w}")
                    nc.tensor.matmul(psw[:], KT_sb[0:64, 0, 0:P],
                                     QT_sb[0:64, 0, 0:TC],
                                     start=True, stop=True)
                # DMA issue costs ~0.6us of sequencer time per dma_start, so
                # spread the issues across the three idle sequencers (SP,
                # ACT, GpSimd) instead of serializing them all on SP.
                nc.sync.dma_start(wk_sb[:], wk)
                nc.scalar.dma_start(wq_sb[:], wq)
                nc.scalar.dma_start(bq_sb[:], bq)
                nc.scalar.dma_start(bk_sb[:], bk)
                for ec in range(NEC):
                    eng = (nc.sync, nc.gpsimd, nc.scalar)[ec % 3]
                    eng.dma_start(hT_sb[:, ec, :], hT[ec])
                nc.gpsimd.dma_start(wv_sb[:], wv)
                nc.gpsimd.dma_start(wo_sb[:], wo)
                nc.sync.dma_start(on2_sb[:], on2)
                nc.sync.dma_start(sel2_sb[:], sel2)

            def proj_qk(w_sb, b_sb, dst, pr, tch):
                """One [128, TC] chunk of the Q or K projection (+bias)."""
                ps = ps_misc.tile([P, TC], F32, tag="ps_misc")
                for ec in range(NEC):
                    nc.tensor.matmul(
                        ps[:], w_sb[:, pr, ec, :],
                        hT_sb[:, ec, tch * TC:(tch + 1) * TC],
                        start=(ec == 0), stop=(ec == NEC - 1))
                nc.vector.tensor_scalar(
                    dst[:, pr, tch * TC:(tch + 1) * TC], ps[:],
                    b_sb[:, pr:pr + 1], None, ADD)

            def proj_v(st):
                """V rows for s-tile st: [128 t, 256 d] -> V_sb bf16."""
                ps = ps_misc.tile([P, TC], F32, tag="ps_misc")
                psv = ps[:, :256]
                for ec in range(NEC):
                    nc.tensor.matmul(
                        psv, hT_sb[:, ec, st * P:(st + 1) * P],
                        wv_sb[:, ec, :], start=(ec == 0), stop=(ec == NEC - 1))
                nc.vector.tensor_copy(V_sb[:, st, :], psv)

            def outproj(tt):
                """Output projection for t-tile tt (128 t) -> HBM."""
                o_sb = work.tile([P, 2 * TC], F32, tag="o")
                for ec2 in range(2):
                    pso = ps_misc.tile([P, TC], F32, tag="ps_misc")
                    for pr in range(2):
                        nc.tensor.matmul(
                            pso[:], ctx_sb[:, pr, tt * P:(tt + 1) * P],
                            wo_sb[:, pr, ec2 * TC:(ec2 + 1) * TC],
                            start=(pr == 0), stop=(pr == 1))
                    nc.vector.tensor_copy(
                        o_sb[:, ec2 * TC:(ec2 + 1) * TC], pso[:])
                eng = nc.gpsimd if tt % 2 else nc.sync
                eng.dma_start(out[tt * P:(tt + 1) * P, :], o_sb[:])

            def pv(pctx, ex, st, pr):
                c0 = pr * P
                nc.tensor.matmul(
                    pctx[0:64, :], V_sb[:, st, c0:c0 + 64], ex[:, :TC],
                    start=(st == 0), stop=(st == NST - 1),
                    tile_position=(0, 0), skip_group_check=True)
                nc.tensor.matmul(
                    pctx[64:P, :], V_sb[:, st, c0 + 64:c0 + P], ex[:, TC:],
                    start=(st == 0), stop=(st == NST - 1),
                    tile_position=(0, 64), skip_group_check=True)

            with nc.named_scope("qkv"):
                # K(t0)/Q(t0) interleaved per e-chunk: each pair of matmuls
                # only needs hT chunk ec, so the PE tracks the chunk DMAs
                psk = ps_misc.tile([P, TC], F32, tag="ps_misc", name="psk0")
                psq = ps_misc.tile([P, TC], F32, tag="ps_misc", name="psq0")
                for ec in range(NEC):
                    nc.tensor.matmul(
                        psk[:], wk_sb[:, 0, ec, :], hT_sb[:, ec, 0:TC],
                        start=(ec == 0), stop=(ec == NEC - 1),
                        skip_group_check=True)
                    nc.tensor.matmul(
                        psq[:], wq_sb[:, 0, ec, :], hT_sb[:, ec, 0:TC],
                        start=(ec == 0), stop=(ec == NEC - 1),
                        skip_group_check=True)
                nc.vector.tensor_scalar(
                    KT_sb[:, 0, 0:TC], psk[:], bk_sb[:, 0:1], None, ADD)
                nc.vector.tensor_scalar(
                    QT_sb[:, 0, 0:TC], psq[:], bq_sb[:, 0:1], None, ADD)

            # work quanta interleaved one-per-iteration into the attention
            # groups as (min_iteration, fn); emission order = semantic order
            # AND scheduler priority.  K(t1..3) are just-in-time inside
            # group 0 (scores of s-tile 4*t need K(t), first used at
            # iteration 4*t); V is split across groups 0-1 ahead of the
            # lag-4 PV consumers; outproj(tt of tch) waits for the ctx
            # scale of group 4+tch, spilled to iteration 6 of group 5+tch.
            def QK(wb, bb, dstb, pr, t):
                return lambda: proj_qk(wb, bb, dstb, pr, t)

            extras = {
                0: [(0, QK(wk_sb, bk_sb, KT_sb, 0, 1)),
                    (1, QK(wk_sb, bk_sb, KT_sb, 0, 2)),
                    (2, QK(wk_sb, bk_sb, KT_sb, 0, 3)),
                    (3, QK(wq_sb, bq_sb, QT_sb, 0, 1))]
                   + [(4 + j, (lambda st=st: proj_v(st)))
                      for j, st in enumerate(range(0, 12))],
                1: [(j, (lambda st=st: proj_v(st)))
                    for j, st in enumerate(range(12, NST))]
                   + [(4, QK(wq_sb, bq_sb, QT_sb, 0, 2)),
                      (8, QK(wk_sb, bk_sb, KT_sb, 1, 0))],
                2: [(0, QK(wq_sb, bq_sb, QT_sb, 0, 3)),
                    (4, QK(wk_sb, bk_sb, KT_sb, 1, 1)),
                    (8, QK(wk_sb, bk_sb, KT_sb, 1, 2))],
                3: [(0, QK(wk_sb, bk_sb, KT_sb, 1, 3)),
                    (6, QK(wq_sb, bq_sb, QT_sb, 1, 0))],
                4: [(0, QK(wq_sb, bq_sb, QT_sb, 1, 1))],
                5: [(0, QK(wq_sb, bq_sb, QT_sb, 1, 2))]
                   + [(7 + 2 * j, (lambda tt=tt: outproj(tt)))
                      for j, tt in enumerate(range(0, 4))],
                6: [(0, QK(wq_sb, bq_sb, QT_sb, 1, 3))]
                   + [(7 + 2 * j, (lambda tt=tt: outproj(tt)))
                      for j, tt in enumerate(range(4, 8))],
                7: [(7 + 2 * j, (lambda tt=tt: outproj(tt)))
                    for j, tt in enumerate(range(8, 12))],
            }

            # ---- attention: 8 groups, pr-outer ----
            # Group 0's PV matmuls and tail are deferred into group 1 so
            # V-proj (interleaved through group 0) has a full group of
            # margin; later groups run PV with a 2-iteration lag.
            def tail_pieces(gi, tch, pr, pctx, rs):
                t0 = tch * TC
                state = {}

                def p_rowsum():
                    prs = ps_misc.tile([P, TC], F32, tag="ps_misc",
                                       name=f"prs_{gi}")
                    nc.tensor.matmul(prs[0:2, :], on2_sb[:, 0:2], rs[:, :TC],
                                     start=True, stop=False)
                    nc.tensor.matmul(prs[0:2, :], on2_sb[:, 2:4], rs[:, TC:],
                                     start=False, stop=True)
                    rcp = rcpool.tile([2, TC], F32, tag="rcp")
                    nc.vector.reciprocal_approx_fast(rcp[:], prs[0:2, :])
                    rcp_bf = rcpool.tile([2, TC], BF16, tag="rcpb")
                    nc.vector.tensor_copy(rcp_bf[:], rcp[:])
                    state["rcp_bf"] = rcp_bf

                def p_bcast():
                    pR = ps_misc.tile([P, TC], F32, tag="ps_misc",
                                      name=f"pR_{gi}")
                    nc.tensor.matmul(pR[:], sel2_sb[:], state["rcp_bf"][:],
                                     start=True, stop=True)
                    state["pR"] = pR

                def p_mult():
                    R_sb = work.tile([P, TC], F32, tag="R", name=f"R_{gi}")
                    nc.vector.tensor_copy(R_sb[:], state["pR"][:])
                    nc.vector.tensor_tensor(
                        ctx_sb[:, pr, t0:t0 + TC], pctx[:], R_sb[:], MULT)

                return [p_rowsum, p_bcast, p_mult]

            with nc.named_scope("attn"):
                groups = [(tch, pr) for pr in range(2) for tch in range(NTC)]
                LAG = 4
                spill = []   # prev group leftovers: PV 12..15 + tail pieces
                for gi, (tch, pr) in enumerate(groups):
                    t0 = tch * TC
                    quota = sorted(extras.get(gi, []), key=lambda x: x[0])
                    qi = 0
                    pctx = ps_ctx.tile([P, TC], F32, tag="ps_ctx")
                    rs = rspool.tile([P, 2 * TC], BF16, tag="rs",
                                     name=f"rs_{gi}")
                    exs = [None] * NST
                    for st in range(NST):
                        s0 = st * P
                        pss = ps_s.tile([P, 2 * TC], F32, tag="ps_s")
                        nc.tensor.matmul(
                            pss[:, :TC], KT_sb[0:64, pr, s0:s0 + P],
                            QT_sb[0:64, pr, t0:t0 + TC],
                            start=True, stop=True, tile_position=(0, 0))
                        nc.tensor.matmul(
                            pss[:, TC:], KT_sb[64:P, pr, s0:s0 + P],
                            QT_sb[64:P, pr, t0:t0 + TC],
                            start=True, stop=True, tile_position=(64, 0))
                        ex = expool.tile([P, 2 * TC], BF16, tag="expT")
                        exs[st] = ex
                        nc.scalar.activation(
                            ex[:], pss[:],
                            mybir.ActivationFunctionType.Exp, scale=0.125)
                        if st == 1:
                            nc.vector.tensor_tensor(
                                rs[:], exs[0][:], ex[:], ADD)
                        elif st > 1:
                            nc.vector.tensor_tensor(rs[:], rs[:], ex[:], ADD)
                        # one extra-work quantum per iteration
                        while qi < len(quota) and quota[qi][0] <= st:
                            quota[qi][1]()
                            qi += 1
                            break
                        # one prev-group spill op per iteration
                        if st < len(spill):
                            spill[st]()
                        # this group's PV, lagged so its exp wait and V-proj
                        # are long satisfied when the PE reaches it
                        if st >= LAG:
                            pv(pctx, exs[st - LAG], st - LAG, pr)
                    while qi < len(quota):
                        quota[qi][1]()
                        qi += 1
                    spill = [
                        (lambda s=s, pc=pctx, e=exs[s], p=pr: pv(pc, e, s, p))
                        for s in range(NST - LAG, NST)
                    ] + tail_pieces(gi, tch, pr, pctx, rs)
                # last group's leftovers
                for fn in spill:
                    fn()

            with nc.named_scope("outproj"):
                for tt in range(12, NST):
                    outproj(tt)
    nc.compile()
    return nc


_NC = None


def _get_nc():
    global _NC
    if _NC is None:
        _NC = build_kernel()
    return _NC


def make_in_maps(hidden_states, Wq, bq, Wk, bk, Wv, bv, Wo, bo, gate):
    f = np.float32
    b16 = ml_dtypes.bfloat16
    hidden_states = np.asarray(hidden_states, f)
    Wq, bq = np.asarray(Wq, f), np.asarray(bq, f)
    Wk, bk = np.asarray(Wk, f), np.asarray(bk, f)
    Wv, bv = np.asarray(Wv, f), np.asarray(bv, f)
    Wo, bo = np.asarray(Wo, f), np.asarray(bo, f)
    gate = np.asarray(gate, f)

    hT_b = [np.ascontiguousarray(hidden_states[b].T)
            .reshape(NEC, P, T).astype(b16) for b in range(B)]
    on2_np = np.zeros((P, 4), b16)
    on2_np[:, 0] = 1.0   # head-A rowsum -> psum row 0
    on2_np[:, 3] = 1.0   # head-B rowsum -> psum row 1
    sel2_np = np.zeros((2, P), b16)
    sel2_np[0, 0:64] = 1.0
    sel2_np[1, 64:P] = 1.0

    in_maps = []
    consts = []
    for core in range(NCORES):
        b, hg = divmod(core, 4)
        hs = [4 * hg + i for i in range(4)]

        def pack_qk(W):
            outw = np.empty((P, 2, NEC, P), f)
            for pr in range(2):
                pair = np.concatenate(
                    [W[hs[2 * pr]], W[hs[2 * pr + 1]]], axis=1)  # [E, 128]
                outw[:, pr] = pair.reshape(NEC, P, P).transpose(1, 0, 2)
            return outw.astype(b16)

        wv_np = np.concatenate([Wv[h] for h in hs], axis=1)  # [E, 256]
        wv_np = wv_np.reshape(NEC, P, 256).transpose(1, 0, 2).astype(b16)
        wo_np = np.empty((2, P, E), f)
        bq_np = np.empty((P, 2), f)
        bk_np = np.empty((P, 2), f)
        for pr in range(2):
            h0, h1 = hs[2 * pr], hs[2 * pr + 1]
            wo_np[pr] = np.concatenate(
                [gate[h0] * Wo[h0], gate[h1] * Wo[h1]], axis=0)  # [128, E]
            bq_np[:, pr] = np.concatenate([bq[h0], bq[h1]])
            bk_np[:, pr] = np.concatenate([bk[h0], bk[h1]])
        # constant term: sum_h gate_h * (bo_h + bv_h @ Wo_h)   [E]
        cst = sum(gate[h] * (bo[h] + bv[h] @ Wo[h]) for h in hs)
        consts.append(np.asarray(cst, f))
        in_maps.append(dict(
            hT=np.ascontiguousarray(hT_b[b]),
            wq=np.ascontiguousarray(pack_qk(Wq)),
            wk=np.ascontiguousarray(pack_qk(Wk)),
            wv=np.ascontiguousarray(wv_np),
            wo=np.ascontiguousarray(wo_np.transpose(1, 0, 2).astype(b16)),
            bq=bq_np, bk=bk_np,
            on2=on2_np, sel2=sel2_np,
        ))
    return in_maps, consts


def kernel(hidden_states, Wq, bq, Wk, bk, Wv, bv, Wo, bo, gate, _trace=False,
           **run_kwargs):
    nc = _get_nc()
    in_maps, consts = make_in_maps(
        hidden_states, Wq, bq, Wk, bk, Wv, bv, Wo, bo, gate)
    res = bass_utils.run_bass_kernel_spmd(
        nc, in_maps, core_ids=list(range(NCORES)), trace=_trace, **run_kwargs)
    outs = [r["out"] for r in res.results]
    full = np.stack([
        outs[0] + outs[1] + outs[2] + outs[3]
        + (consts[0] + consts[1] + consts[2] + consts[3])[None, :],
        outs[4] + outs[5] + outs[6] + outs[7]
        + (consts[4] + consts[5] + consts[6] + consts[7])[None, :],
    ]).astype(np.float32)
    kernel.last_result = res
    return full
